# revision 25
# baseline (speedup 1.0000x reference)
"""Trainium2 Bass kernel for single-head cross-modal attention.

Problem: B=8, S=2048, D=1024 (fp32 inputs)
    q = image_emb @ Wq.T + bq
    k = text_emb  @ Wk.T + bk
    v = text_emb  @ Wv.T + bv
    out = softmax(q @ k.T / sqrt(D)) @ v
Sharding: data-parallel over batch — core b handles batch element b.

Key algebraic restructure (kills the on-device K projection):
    q k^T = Xi (Wq^T Wk) Xt^T + (per-query-row constants) + 1 (bq^T Wk) Xt^T
Softmax is row-shift invariant, so the row-constant terms drop. With
host-precomputed M = Wq^T Wk and c = bq @ Wk:
    scores ~ (Xi M + 1 c^T) Xt^T
The Q projection becomes A = Xi M + c (same cost, bias reused) and K^T
is just Xt^T — DMA'd once into SBUF and reused both as the scores
stationary and as the V-projection stationary.

DMA strategy: every dma_start costs ~0.7us on the issuing sequencer, so
all bulk tensors are host-relayouted to SBUF-tile order and shipped as
a handful of large contiguous transfers, ordered by first use so the
PE starts ~2us after the NEFF preamble.

Per-core dataflow (matmuls bf16, fp32 PSUM):
  - AT[d',q] per 512-col chunk: stationary M[d,d'_tile], moving XiT.
  - scoresT[k,q] = kt_tile.T @ AT -> exp -> stationary of P@V: the
    2048x2048 probability matrix is never transposed on chip.
  - no-max softmax (scores ~ N(0,1)); row sums via ones-column matmul;
    final normalize fused with bv add in one DVE op per 512-chunk.
"""

import sys
import os

for _p in ("/opt/trn_rl_repo", "/root/.axon_site/_ro/trn_rl_repo"):
    if os.path.isdir(_p) and _p not in sys.path:
        sys.path.insert(0, _p)

import numpy as np
import ml_dtypes

import concourse.bass as bass
import concourse.mybir as mybir
import concourse.tile as tile
from concourse import bacc
from concourse.bass_utils import run_bass_kernel_spmd

BF16 = mybir.dt.bfloat16
F32 = mybir.dt.float32
AF = mybir.ActivationFunctionType
ALU = mybir.AluOpType

B, S, D = 8, 2048, 1024
P = 128
ND = D // P          # 8  d tiles
NS = S // P          # 16 s tiles
QC = 512             # q chunk width (matmul free dim / PSUM bank)
NQC = S // QC        # 4
EC = 512             # e chunk width for V / output
SCALE = 1.0 / float(np.sqrt(D))

_CACHE = {}


def _build_nc():
    nc = bacc.Bacc("TRN2", target_bir_lowering=False, debug=False, num_devices=8)

    # all bulk tensors are pre-relayouted on host to SBUF tile order
    xi_d = nc.dram_tensor("xi", [P, NQC, ND * QC], BF16, kind="ExternalInput").ap()
    xt_d = nc.dram_tensor("xt", [P, ND, S], BF16, kind="ExternalInput").ap()
    m_d = nc.dram_tensor("m", [P, ND, ND * P], BF16, kind="ExternalInput").ap()
    wvt_d = nc.dram_tensor("wvt", [P, ND, D], BF16, kind="ExternalInput").ap()
    ca_d = nc.dram_tensor("ca", [P, ND], F32, kind="ExternalInput").ap()
    bv_d = nc.dram_tensor("bv", [D], F32, kind="ExternalInput").ap()
    out_d = nc.dram_tensor("out", [S, D], F32, kind="ExternalOutput").ap()

    with tile.TileContext(nc) as tc:
        _emit(nc, tc, xi_d, xt_d, m_d, wvt_d, ca_d, bv_d, out_d)
    nc.compile()
    return nc


def _emit(nc, tc, xi_d, xt_d, m_d, wvt_d, ca_d, bv_d, out_d):
    NH = QC // P  # 4 q_tiles per chunk
    with (
        tc.tile_pool(name="const", bufs=1) as pc,
        tc.tile_pool(name="qkv", bufs=1) as pqkv,
    ):
        # persistent activations
        at = pqkv.tile([P, ND, S], BF16, name="at", tag="at")    # AT[d',q]
        kt = pqkv.tile([P, ND, S], BF16, name="kt", tag="kt")    # XtT[d',k]
        v = pqkv.tile([P, NS, D], BF16, name="v", tag="v")       # V[s,e]

        # constants
        bias_a = pc.tile([P, ND], F32, name="bias_a", tag="bias_a")
        ones_row = pc.tile([1, P], F32, name="ones_row", tag="ones_row")
        bv_row = pc.tile([1, D], F32, name="bv_row", tag="bv_row")
        bv_bcast = pc.tile([P, D], F32, name="bv_bcast", tag="bv_bcast")
        ones_col = pc.tile([P, 1], BF16, name="ones_col", tag="ones_col")

        with (
            tc.tile_pool(name="w", bufs=1) as pw,
            tc.tile_pool(name="xs", bufs=1) as pxs,
            tc.tile_pool(name="psP", bufs=6, space="PSUM") as psP,
        ):
            # m_sb[:, et, d*P:(d+1)*P] = M[d-block, et-block] (et-major!)
            m_sb = pw.tile([P, ND, ND * P], BF16, name="m_sb", tag="m_sb")
            wv_sb = pw.tile([P, ND, D], BF16, name="wv_sb", tag="wv_sb")
            # xc[:, qc, d*QC:(d+1)*QC] = XiT[d-block, qc-chunk]
            xc = pxs.tile([P, NQC, ND * QC], BF16, name="xc", tag="xs")

            # --- bulk DMAs: few triggers, ordered by first use.
            # Ring order is per-engine FIFO; active rings share the 16 SDMA
            # engines, so first-needed pieces are kept small and bulk (kt,
            # wv) is queued last on each ring.
            # NOTE: gpsimd triggers lower to qPoolDynamic (the VECTOR
            # engine's stream, slow SW completions) — avoid it entirely;
            # everything rides the two HWDGE rings (sync/scalar).
            nc.scalar.dma_start(bv_row[:], bv_d[None, :])         # 4KB
            nc.sync.dma_start(bias_a[:], ca_d[:])                 # 4KB
            nc.sync.dma_start(m_sb[:, 0, 0:4 * P], m_d[:, 0, 0:4 * P])  # 128K
            nc.scalar.dma_start(xc[:, 0, 0:4 * QC], xi_d[:, 0, 0:4 * QC])
            nc.sync.dma_start(m_sb[:, 0, 4 * P:], m_d[:, 0, 4 * P:])
            nc.scalar.dma_start(xc[:, 0, 4 * QC:], xi_d[:, 0, 4 * QC:])
            nc.sync.dma_start(m_sb[:, 1, :], m_d[:, 1, :])
            nc.sync.dma_start(m_sb[:, 2:5, :], m_d[:, 2:5, :])    # 768KB
            nc.sync.dma_start(m_sb[:, 5:8, :], m_d[:, 5:8, :])
            nc.scalar.dma_start(xc[:, 1, :], xi_d[:, 1, :])
            nc.scalar.dma_start(xc[:, 2, :], xi_d[:, 2, :])
            nc.scalar.dma_start(xc[:, 3, :], xi_d[:, 3, :])
            # bulk not needed until the V phase (~70us), at the ring tails
            nc.sync.dma_start(wv_sb[:], wvt_d[:])                 # 2MB
            nc.sync.dma_start(kt[:, 0:4, :], xt_d[:, 0:4, :])
            nc.scalar.dma_start(kt[:, 4:8, :], xt_d[:, 4:8, :])
            nc.vector.memset(ones_row[:], 1.0)
            nc.vector.memset(ones_col[:], 1.0)

            # bv broadcast first: needs only bv_row + memset, warms the PE
            for c in range(2):
                pb = psP.tile([P, EC], F32, name="pb", tag="ps")
                nc.tensor.matmul(
                    pb[:], ones_row[:], bv_row[:, c * EC:(c + 1) * EC],
                    start=True, stop=True)
                nc.vector.tensor_copy(bv_bcast[:, c * EC:(c + 1) * EC], pb[:])

            # --- AT[d'_t, qc] = sum_d M[d, d'_t].T @ XiT[d, qc]  (+ c) ---
            for qc in range(NQC):
                for et in range(ND):
                    ps = psP.tile([P, QC], F32, name="ps", tag="ps")
                    for d in range(ND):
                        nc.tensor.matmul(
                            ps[:], m_sb[:, et, d * P:(d + 1) * P],
                            xc[:, qc, d * QC:(d + 1) * QC],
                            start=(d == 0), stop=(d == ND - 1))
                    nc.vector.tensor_scalar_add(
                        at[:, et, qc * QC:(qc + 1) * QC], ps[:],
                        bias_a[:, et:et + 1])

            # --- V[s_t, e] = sum_d XtT[d, s_t].T @ Wv^T[d, e] ---
            for vc in range(NQC):
                for si in range(NH):
                    st = vc * NH + si
                    ps0 = psP.tile([P, EC], F32, name="ps0", tag="ps")
                    ps1 = psP.tile([P, EC], F32, name="ps1", tag="ps")
                    for d in range(ND):
                        lhs = kt[:, d, st * P:(st + 1) * P]
                        nc.tensor.matmul(ps0[:], lhs, wv_sb[:, d, 0:EC],
                                         start=(d == 0), stop=(d == ND - 1))
                        nc.tensor.matmul(ps1[:], lhs, wv_sb[:, d, EC:D],
                                         start=(d == 0), stop=(d == ND - 1))
                    nc.vector.tensor_copy(v[:, st, 0:EC], ps0[:])
                    nc.vector.tensor_copy(v[:, st, EC:D], ps1[:])

        # --- attention ---
        with (
            tc.tile_pool(name="et", bufs=3) as pet,
            tc.tile_pool(name="outp", bufs=2) as pout,
            tc.tile_pool(name="stat", bufs=4) as pstat,
            tc.tile_pool(name="psST", bufs=2, space="PSUM") as psST,
            tc.tile_pool(name="psAV", bufs=4, space="PSUM") as psAV,
            tc.tile_pool(name="psRS", bufs=2, space="PSUM") as psRS,
        ):
            for qc in range(NQC):
                # scores^T for this q chunk: ET[kk, q] = exp(scale*XtT.T@AT)
                et_t = pet.tile([P, NS, QC], BF16, name="et_t", tag="et")
                for kk in range(NS):
                    st_ps = psST.tile([P, QC], F32, name="st_ps", tag="st")
                    for e in range(ND):
                        nc.tensor.matmul(
                            st_ps[:],
                            kt[:, e, kk * P:(kk + 1) * P],
                            at[:, e, qc * QC:(qc + 1) * QC],
                            start=(e == 0), stop=(e == ND - 1))
                    nc.scalar.activation(et_t[:, kk, :], st_ps[:], AF.Exp,
                                         scale=SCALE)

                # attended[q_t, :] = (ET.T @ V) * recip + bv
                for qs in range(NH):
                    q_tile = qc * NH + qs
                    last = q_tile == NS - 1
                    a0 = psAV.tile([P, EC], F32, name="a0", tag="av")
                    a1 = psAV.tile([P, EC], F32, name="a1", tag="av")
                    rs = psRS.tile([P, 1], F32, name="rs", tag="rs")
                    recip = pstat.tile([P, 1], F32, name="recip", tag="recip")
                    ob = pout.tile([P, D], F32, name="ob", tag="ob")
                    if not last:
                        for kk in range(NS):
                            lhs = et_t[:, kk, qs * P:(qs + 1) * P]
                            nc.tensor.matmul(a0[:], lhs, v[:, kk, 0:EC],
                                             start=(kk == 0),
                                             stop=(kk == NS - 1))
                            nc.tensor.matmul(a1[:], lhs, v[:, kk, EC:D],
                                             start=(kk == 0),
                                             stop=(kk == NS - 1))
                            nc.tensor.matmul(rs[:], lhs, ones_col[:],
                                             start=(kk == 0),
                                             stop=(kk == NS - 1))
                        nc.vector.reciprocal(recip[:], rs[:])
                        nc.vector.scalar_tensor_tensor(
                            ob[:, 0:EC], a0[:], recip[:], bv_bcast[:, 0:EC],
                            op0=ALU.mult, op1=ALU.add)
                        nc.vector.scalar_tensor_tensor(
                            ob[:, EC:D], a1[:], recip[:], bv_bcast[:, EC:D],
                            op0=ALU.mult, op1=ALU.add)
                        eng = nc.sync if q_tile % 2 == 0 else nc.scalar
                        eng.dma_start(
                            out_d[q_tile * P:(q_tile + 1) * P, :], ob[:])
                    else:
                        # final tile: finish cols EC:D (+rowsum) first so
                        # their combine+DMA overlap the cols 0:EC chain —
                        # the tail after the last matmul is one 256KB DMA.
                        for kk in range(NS):
                            lhs = et_t[:, kk, qs * P:(qs + 1) * P]
                            nc.tensor.matmul(a1[:], lhs, v[:, kk, EC:D],
                                             start=(kk == 0),
                                             stop=(kk == NS - 1))
                            nc.tensor.matmul(rs[:], lhs, ones_col[:],
                                             start=(kk == 0),
                                             stop=(kk == NS - 1))
                        nc.vector.reciprocal(recip[:], rs[:])
                        nc.vector.scalar_tensor_tensor(
                            ob[:, EC:D], a1[:], recip[:], bv_bcast[:, EC:D],
                            op0=ALU.mult, op1=ALU.add)
                        nc.scalar.dma_start(
                            out_d[q_tile * P:(q_tile + 1) * P, EC:D],
                            ob[:, EC:D])
                        for kk in range(NS):
                            lhs = et_t[:, kk, qs * P:(qs + 1) * P]
                            nc.tensor.matmul(a0[:], lhs, v[:, kk, 0:EC],
                                             start=(kk == 0),
                                             stop=(kk == NS - 1))
                        nc.vector.scalar_tensor_tensor(
                            ob[:, 0:EC // 2], a0[:, 0:EC // 2], recip[:],
                            bv_bcast[:, 0:EC // 2],
                            op0=ALU.mult, op1=ALU.add)
                        nc.sync.dma_start(
                            out_d[q_tile * P:(q_tile + 1) * P, 0:EC // 2],
                            ob[:, 0:EC // 2])
                        nc.vector.scalar_tensor_tensor(
                            ob[:, EC // 2:EC], a0[:, EC // 2:EC], recip[:],
                            bv_bcast[:, EC // 2:EC],
                            op0=ALU.mult, op1=ALU.add)
                        nc.scalar.dma_start(
                            out_d[q_tile * P:(q_tile + 1) * P, EC // 2:EC],
                            ob[:, EC // 2:EC])


def get_nc():
    if "nc" not in _CACHE:
        _CACHE["nc"] = _build_nc()
    return _CACHE["nc"]


def _prep_inputs(image_emb, text_emb, Wq, bq, Wk, bk, Wv, bv):
    bf = ml_dtypes.bfloat16
    xi = np.asarray(image_emb)   # [B, S, D] f32
    xt = np.asarray(text_emb)
    wq = np.asarray(Wq, dtype=np.float32)
    wk = np.asarray(Wk, dtype=np.float32)

    # m host layout [P, ND(et), ND(d)*P]: m[p, et, d*P+c] = M[d*P+p, et*P+c]
    m = (wq.T @ wk).astype(bf)                       # [D, D]
    m = m.reshape(ND, P, ND, P).transpose(1, 2, 0, 3).reshape(P, ND, ND * P)
    m = np.ascontiguousarray(m)

    ca = np.asarray(bq, dtype=np.float32) @ wk       # [D]
    ca = np.ascontiguousarray(ca.reshape(ND, P).T)   # [P, ND]

    # wvt [P, ND(d), D(e)]: wvt[p, d, e] = Wv[e, d*P+p]
    wvt = np.asarray(Wv).T.astype(bf).reshape(ND, P, D).transpose(1, 0, 2)
    wvt = np.ascontiguousarray(wvt)

    # xt [B, P, ND(d), S]: XtT tile order
    xtT = xt.transpose(0, 2, 1).astype(bf)           # [B, D, S]
    xtr = np.ascontiguousarray(
        xtT.reshape(B, ND, P, S).transpose(0, 2, 1, 3))

    # xi [B, P, NQC, ND*QC]: xi[b, p, qc, d*QC+c] = XiT[b, d*P+p, qc*QC+c]
    xiT = xi.transpose(0, 2, 1).astype(bf)           # [B, D, S]
    xir = np.ascontiguousarray(
        xiT.reshape(B, ND, P, NQC, QC).transpose(0, 2, 3, 1, 4)
        .reshape(B, P, NQC, ND * QC))

    bv = np.asarray(bv, dtype=np.float32)
    in_maps = []
    for b in range(B):
        in_maps.append({
            "xi": xir[b], "xt": xtr[b],
            "m": m, "wvt": wvt, "ca": ca, "bv": bv,
        })
    return in_maps


def run(image_emb, text_emb, Wq, bq, Wk, bk, Wv, bv, trace=False, **spmd_kwargs):
    nc = get_nc()
    in_maps = _prep_inputs(image_emb, text_emb, Wq, bq, Wk, bk, Wv, bv)
    res = run_bass_kernel_spmd(nc, in_maps, list(range(B)), trace=trace,
                               **spmd_kwargs)
    out = np.stack([res.results[b]["out"] for b in range(B)], axis=0)
    return out, res


def kernel(image_emb, text_emb, edge_index=None, Wq=None, bq=None, Wk=None,
           bk=None, Wv=None, bv=None, **_unused):
    out, _ = run(image_emb, text_emb, Wq, bq, Wk, bk, Wv, bv, trace=False)
    return out


# revision 31
# speedup vs baseline: 1.0095x; 1.0095x over previous
"""Trainium2 Bass kernel for single-head cross-modal attention.

Problem: B=8, S=2048, D=1024 (fp32 inputs)
    q = image_emb @ Wq.T + bq
    k = text_emb  @ Wk.T + bk
    v = text_emb  @ Wv.T + bv
    out = softmax(q @ k.T / sqrt(D)) @ v
Sharding: data-parallel over batch — core b handles batch element b.

Key algebraic restructure (kills the on-device K projection):
    q k^T = Xi (Wq^T Wk) Xt^T + (per-query-row constants) + 1 (bq^T Wk) Xt^T
Softmax is row-shift invariant, so the row-constant terms drop. With
host-precomputed M = Wq^T Wk and c = bq @ Wk:
    scores ~ (Xi M + 1 c^T) Xt^T
The Q projection becomes A = Xi M + c (same cost, bias reused) and K^T
is just Xt^T — DMA'd once into SBUF and reused both as the scores
stationary and as the V-projection stationary.

DMA strategy: every dma_start costs ~0.7us on the issuing sequencer, so
all bulk tensors are host-relayouted to SBUF-tile order and shipped as
a handful of large contiguous transfers, ordered by first use so the
PE starts ~2us after the NEFF preamble.

Per-core dataflow (matmuls bf16, fp32 PSUM):
  - AT[d',q] per 512-col chunk: stationary M[d,d'_tile], moving XiT.
  - scoresT[k,q] = kt_tile.T @ AT -> exp -> stationary of P@V: the
    2048x2048 probability matrix is never transposed on chip.
  - no-max softmax (scores ~ N(0,1)); row sums via ones-column matmul;
    final normalize fused with bv add in one DVE op per 512-chunk.
"""

import sys
import os

for _p in ("/opt/trn_rl_repo", "/root/.axon_site/_ro/trn_rl_repo"):
    if os.path.isdir(_p) and _p not in sys.path:
        sys.path.insert(0, _p)

import numpy as np
import ml_dtypes

import concourse.bass as bass
import concourse.mybir as mybir
import concourse.tile as tile
from concourse import bacc
from concourse.bass_utils import run_bass_kernel_spmd

BF16 = mybir.dt.bfloat16
F32 = mybir.dt.float32
AF = mybir.ActivationFunctionType
ALU = mybir.AluOpType

B, S, D = 8, 2048, 1024
P = 128
ND = D // P          # 8  d tiles
NS = S // P          # 16 s tiles
QC = 512             # q chunk width (matmul free dim / PSUM bank)
NQC = S // QC        # 4
EC = 512             # e chunk width for V / output
SCALE = 1.0 / float(np.sqrt(D))

_CACHE = {}


def _build_nc():
    nc = bacc.Bacc("TRN2", target_bir_lowering=False, debug=False, num_devices=8)

    # all bulk tensors are pre-relayouted on host to SBUF tile order
    xi_d = nc.dram_tensor("xi", [P, NQC, ND * QC], BF16, kind="ExternalInput").ap()
    xt_d = nc.dram_tensor("xt", [P, ND, S], BF16, kind="ExternalInput").ap()
    m_d = nc.dram_tensor("m", [P, ND, ND * P], BF16, kind="ExternalInput").ap()
    wvt_d = nc.dram_tensor("wvt", [P, ND, D], BF16, kind="ExternalInput").ap()
    ca_d = nc.dram_tensor("ca", [P, ND], F32, kind="ExternalInput").ap()
    bv_d = nc.dram_tensor("bv", [D], F32, kind="ExternalInput").ap()
    out_d = nc.dram_tensor("out", [S, D], F32, kind="ExternalOutput").ap()

    with tile.TileContext(nc) as tc:
        _emit(nc, tc, xi_d, xt_d, m_d, wvt_d, ca_d, bv_d, out_d)
    nc.compile()
    return nc


def _emit(nc, tc, xi_d, xt_d, m_d, wvt_d, ca_d, bv_d, out_d):
    NH = QC // P  # 4 q_tiles per chunk
    with (
        tc.tile_pool(name="const", bufs=1) as pc,
        tc.tile_pool(name="qkv", bufs=1) as pqkv,
    ):
        # persistent activations
        at = pqkv.tile([P, ND, S], BF16, name="at", tag="at")    # AT[d',q]
        kt = pqkv.tile([P, ND, S], BF16, name="kt", tag="kt")    # XtT[d',k]
        # V[s, e] with a ones column appended at e=D: the P@V row sums
        # (softmax denominators) fall out of the third PV chain, so no
        # 1-column rowsum matmuls (whose ldweights never hide) are needed.
        v = pqkv.tile([P, NS, D + 1], BF16, name="v", tag="v")

        # constants
        bias_a = pc.tile([P, ND], F32, name="bias_a", tag="bias_a")
        ones_row = pc.tile([1, P], F32, name="ones_row", tag="ones_row")
        bv_row = pc.tile([1, D], F32, name="bv_row", tag="bv_row")
        bv_bcast = pc.tile([P, D], F32, name="bv_bcast", tag="bv_bcast")


        with (
            tc.tile_pool(name="w", bufs=1) as pw,
            tc.tile_pool(name="xs", bufs=1) as pxs,
            tc.tile_pool(name="psP", bufs=6, space="PSUM") as psP,
        ):
            # m_sb[:, et, d*P:(d+1)*P] = M[d-block, et-block] (et-major!)
            m_sb = pw.tile([P, ND, ND * P], BF16, name="m_sb", tag="m_sb")
            wv_sb = pw.tile([P, ND, D], BF16, name="wv_sb", tag="wv_sb")
            # xc[:, qc, d*QC:(d+1)*QC] = XiT[d-block, qc-chunk]
            xc = pxs.tile([P, NQC, ND * QC], BF16, name="xc", tag="xs")

            # --- bulk DMAs: few triggers, ordered by first use.
            # Ring order is per-engine FIFO; active rings share the 16 SDMA
            # engines, so first-needed pieces are kept small and bulk (kt,
            # wv) is queued last on each ring.
            # NOTE: gpsimd triggers lower to qPoolDynamic (the VECTOR
            # engine's stream, slow SW completions) — avoid it entirely;
            # everything rides the two HWDGE rings (sync/scalar).
            nc.scalar.dma_start(bv_row[:], bv_d[None, :])         # 4KB
            nc.sync.dma_start(bias_a[:], ca_d[:])                 # 4KB
            nc.sync.dma_start(m_sb[:, 0, 0:4 * P], m_d[:, 0, 0:4 * P])  # 128K
            nc.scalar.dma_start(xc[:, 0, 0:4 * QC], xi_d[:, 0, 0:4 * QC])
            nc.sync.dma_start(m_sb[:, 0, 4 * P:], m_d[:, 0, 4 * P:])
            nc.scalar.dma_start(xc[:, 0, 4 * QC:], xi_d[:, 0, 4 * QC:])
            nc.sync.dma_start(m_sb[:, 1, :], m_d[:, 1, :])
            nc.sync.dma_start(m_sb[:, 2:5, :], m_d[:, 2:5, :])    # 768KB
            nc.sync.dma_start(m_sb[:, 5:8, :], m_d[:, 5:8, :])
            nc.scalar.dma_start(xc[:, 1, :], xi_d[:, 1, :])
            nc.scalar.dma_start(xc[:, 2, :], xi_d[:, 2, :])
            nc.scalar.dma_start(xc[:, 3, :], xi_d[:, 3, :])
            # bulk not needed until the V phase (~70us), at the ring tails
            nc.sync.dma_start(wv_sb[:], wvt_d[:])                 # 2MB
            nc.sync.dma_start(kt[:, 0:4, :], xt_d[:, 0:4, :])
            nc.scalar.dma_start(kt[:, 4:8, :], xt_d[:, 4:8, :])
            nc.vector.memset(ones_row[:], 1.0)
            nc.vector.memset(v[:, :, D:D + 1], 1.0)

            # bv broadcast first: needs only bv_row + memset, warms the PE
            for c in range(2):
                pb = psP.tile([P, EC], F32, name="pb", tag="ps")
                nc.tensor.matmul(
                    pb[:], ones_row[:], bv_row[:, c * EC:(c + 1) * EC],
                    start=True, stop=True)
                nc.vector.tensor_copy(bv_bcast[:, c * EC:(c + 1) * EC], pb[:])

            # --- AT[d'_t, qc] = sum_d M[d, d'_t].T @ XiT[d, qc]  (+ c) ---
            for qc in range(NQC):
                for et in range(ND):
                    ps = psP.tile([P, QC], F32, name="ps", tag="ps")
                    for d in range(ND):
                        nc.tensor.matmul(
                            ps[:], m_sb[:, et, d * P:(d + 1) * P],
                            xc[:, qc, d * QC:(d + 1) * QC],
                            start=(d == 0), stop=(d == ND - 1))
                    nc.vector.tensor_scalar_add(
                        at[:, et, qc * QC:(qc + 1) * QC], ps[:],
                        bias_a[:, et:et + 1])

            # --- V[s_t, e] = sum_d XtT[d, s_t].T @ Wv^T[d, e] ---
            for vc in range(NQC):
                for si in range(NH):
                    st = vc * NH + si
                    ps0 = psP.tile([P, EC], F32, name="ps0", tag="ps")
                    ps1 = psP.tile([P, EC], F32, name="ps1", tag="ps")
                    for d in range(ND):
                        lhs = kt[:, d, st * P:(st + 1) * P]
                        nc.tensor.matmul(ps0[:], lhs, wv_sb[:, d, 0:EC],
                                         start=(d == 0), stop=(d == ND - 1))
                        nc.tensor.matmul(ps1[:], lhs, wv_sb[:, d, EC:D],
                                         start=(d == 0), stop=(d == ND - 1))
                    nc.vector.tensor_copy(v[:, st, 0:EC], ps0[:])
                    nc.vector.tensor_copy(v[:, st, EC:D], ps1[:])

        # --- attention ---
        with (
            tc.tile_pool(name="et", bufs=3) as pet,
            tc.tile_pool(name="outp", bufs=2) as pout,
            tc.tile_pool(name="stat", bufs=4) as pstat,
            tc.tile_pool(name="psST", bufs=2, space="PSUM") as psST,
            tc.tile_pool(name="psAV", bufs=6, space="PSUM") as psAV,
        ):
            for qc in range(NQC):
                # scores^T for this q chunk: ET[kk, q] = exp(scale*XtT.T@AT)
                et_t = pet.tile([P, NS, QC], BF16, name="et_t", tag="et")
                for kk in range(NS):
                    st_ps = psST.tile([P, QC], F32, name="st_ps", tag="st")
                    for e in range(ND):
                        nc.tensor.matmul(
                            st_ps[:],
                            kt[:, e, kk * P:(kk + 1) * P],
                            at[:, e, qc * QC:(qc + 1) * QC],
                            start=(e == 0), stop=(e == ND - 1))
                    nc.scalar.activation(et_t[:, kk, :], st_ps[:], AF.Exp,
                                         scale=SCALE)

                # attended[q_t, :] = (ET.T @ [V|1]) * recip + bv
                # three chains per q_tile: cols 0:512, 512:768, 768:1025
                # (last includes the ones column => softmax denominators).
                # Every chain streams >=256 rows, so each matmul's
                # ldweights hides under the previous stream.
                C1, C2 = 512, 768
                for qs in range(NH):
                    q_tile = qc * NH + qs
                    last = q_tile == NS - 1
                    a0 = psAV.tile([P, EC], F32, name="a0", tag="av")
                    a1 = psAV.tile([P, EC], F32, name="a1", tag="av")
                    a2 = psAV.tile([P, EC], F32, name="a2", tag="av")
                    recip = pstat.tile([P, 1], F32, name="recip", tag="recip")
                    ob = pout.tile([P, D], F32, name="ob", tag="ob")
                    if not last:
                        for kk in range(NS):
                            lhs = et_t[:, kk, qs * P:(qs + 1) * P]
                            nc.tensor.matmul(a0[:], lhs, v[:, kk, 0:C1],
                                             start=(kk == 0),
                                             stop=(kk == NS - 1))
                            nc.tensor.matmul(a1[:, 0:C2 - C1], lhs,
                                             v[:, kk, C1:C2],
                                             start=(kk == 0),
                                             stop=(kk == NS - 1))
                            nc.tensor.matmul(a2[:, 0:D + 1 - C2], lhs,
                                             v[:, kk, C2:D + 1],
                                             start=(kk == 0),
                                             stop=(kk == NS - 1))
                        nc.vector.reciprocal(recip[:], a2[:, D - C2:D - C2 + 1])
                        nc.vector.scalar_tensor_tensor(
                            ob[:, 0:C1], a0[:], recip[:], bv_bcast[:, 0:C1],
                            op0=ALU.mult, op1=ALU.add)
                        nc.vector.scalar_tensor_tensor(
                            ob[:, C1:C2], a1[:, 0:C2 - C1], recip[:],
                            bv_bcast[:, C1:C2],
                            op0=ALU.mult, op1=ALU.add)
                        nc.vector.scalar_tensor_tensor(
                            ob[:, C2:D], a2[:, 0:D - C2], recip[:],
                            bv_bcast[:, C2:D],
                            op0=ALU.mult, op1=ALU.add)
                        eng = nc.sync if q_tile % 2 == 0 else nc.scalar
                        eng.dma_start(
                            out_d[q_tile * P:(q_tile + 1) * P, :], ob[:])
                    else:
                        # final tile: upper cols (+denominator) first so
                        # their combine+DMA overlap the cols 0:512 chain —
                        # the tail after the last matmul is small.
                        for kk in range(NS):
                            lhs = et_t[:, kk, qs * P:(qs + 1) * P]
                            nc.tensor.matmul(a1[:, 0:C2 - C1], lhs,
                                             v[:, kk, C1:C2],
                                             start=(kk == 0),
                                             stop=(kk == NS - 1))
                            nc.tensor.matmul(a2[:, 0:D + 1 - C2], lhs,
                                             v[:, kk, C2:D + 1],
                                             start=(kk == 0),
                                             stop=(kk == NS - 1))
                        nc.vector.reciprocal(recip[:], a2[:, D - C2:D - C2 + 1])
                        nc.vector.scalar_tensor_tensor(
                            ob[:, C1:C2], a1[:, 0:C2 - C1], recip[:],
                            bv_bcast[:, C1:C2],
                            op0=ALU.mult, op1=ALU.add)
                        nc.vector.scalar_tensor_tensor(
                            ob[:, C2:D], a2[:, 0:D - C2], recip[:],
                            bv_bcast[:, C2:D],
                            op0=ALU.mult, op1=ALU.add)
                        nc.scalar.dma_start(
                            out_d[q_tile * P:(q_tile + 1) * P, C1:D],
                            ob[:, C1:D])
                        for kk in range(NS):
                            lhs = et_t[:, kk, qs * P:(qs + 1) * P]
                            nc.tensor.matmul(a0[:], lhs, v[:, kk, 0:C1],
                                             start=(kk == 0),
                                             stop=(kk == NS - 1))
                        nc.vector.scalar_tensor_tensor(
                            ob[:, 0:C1 // 2], a0[:, 0:C1 // 2], recip[:],
                            bv_bcast[:, 0:C1 // 2],
                            op0=ALU.mult, op1=ALU.add)
                        nc.sync.dma_start(
                            out_d[q_tile * P:(q_tile + 1) * P, 0:C1 // 2],
                            ob[:, 0:C1 // 2])
                        nc.vector.scalar_tensor_tensor(
                            ob[:, C1 // 2:C1], a0[:, C1 // 2:C1], recip[:],
                            bv_bcast[:, C1 // 2:C1],
                            op0=ALU.mult, op1=ALU.add)
                        nc.scalar.dma_start(
                            out_d[q_tile * P:(q_tile + 1) * P, C1 // 2:C1],
                            ob[:, C1 // 2:C1])


def get_nc():
    if "nc" not in _CACHE:
        _CACHE["nc"] = _build_nc()
    return _CACHE["nc"]


def _prep_inputs(image_emb, text_emb, Wq, bq, Wk, bk, Wv, bv):
    bf = ml_dtypes.bfloat16
    xi = np.asarray(image_emb)   # [B, S, D] f32
    xt = np.asarray(text_emb)
    wq = np.asarray(Wq, dtype=np.float32)
    wk = np.asarray(Wk, dtype=np.float32)

    # m host layout [P, ND(et), ND(d)*P]: m[p, et, d*P+c] = M[d*P+p, et*P+c]
    m = (wq.T @ wk).astype(bf)                       # [D, D]
    m = m.reshape(ND, P, ND, P).transpose(1, 2, 0, 3).reshape(P, ND, ND * P)
    m = np.ascontiguousarray(m)

    ca = np.asarray(bq, dtype=np.float32) @ wk       # [D]
    ca = np.ascontiguousarray(ca.reshape(ND, P).T)   # [P, ND]

    # wvt [P, ND(d), D(e)]: wvt[p, d, e] = Wv[e, d*P+p]
    wvt = np.asarray(Wv).T.astype(bf).reshape(ND, P, D).transpose(1, 0, 2)
    wvt = np.ascontiguousarray(wvt)

    # xt [B, P, ND(d), S]: XtT tile order
    xtT = xt.transpose(0, 2, 1).astype(bf)           # [B, D, S]
    xtr = np.ascontiguousarray(
        xtT.reshape(B, ND, P, S).transpose(0, 2, 1, 3))

    # xi [B, P, NQC, ND*QC]: xi[b, p, qc, d*QC+c] = XiT[b, d*P+p, qc*QC+c]
    xiT = xi.transpose(0, 2, 1).astype(bf)           # [B, D, S]
    xir = np.ascontiguousarray(
        xiT.reshape(B, ND, P, NQC, QC).transpose(0, 2, 3, 1, 4)
        .reshape(B, P, NQC, ND * QC))

    bv = np.asarray(bv, dtype=np.float32)
    in_maps = []
    for b in range(B):
        in_maps.append({
            "xi": xir[b], "xt": xtr[b],
            "m": m, "wvt": wvt, "ca": ca, "bv": bv,
        })
    return in_maps


def run(image_emb, text_emb, Wq, bq, Wk, bk, Wv, bv, trace=False, **spmd_kwargs):
    nc = get_nc()
    in_maps = _prep_inputs(image_emb, text_emb, Wq, bq, Wk, bk, Wv, bv)
    res = run_bass_kernel_spmd(nc, in_maps, list(range(B)), trace=trace,
                               **spmd_kwargs)
    out = np.stack([res.results[b]["out"] for b in range(B)], axis=0)
    return out, res


def kernel(image_emb, text_emb, edge_index=None, Wq=None, bq=None, Wk=None,
           bk=None, Wv=None, bv=None, **_unused):
    out, _ = run(image_emb, text_emb, Wq, bq, Wk, bk, Wv, bv, trace=False)
    return out


# revision 32
# speedup vs baseline: 1.0132x; 1.0037x over previous
"""Trainium2 Bass kernel for single-head cross-modal attention.

Problem: B=8, S=2048, D=1024 (fp32 inputs)
    q = image_emb @ Wq.T + bq
    k = text_emb  @ Wk.T + bk
    v = text_emb  @ Wv.T + bv
    out = softmax(q @ k.T / sqrt(D)) @ v
Sharding: data-parallel over batch — core b handles batch element b.

Key algebraic restructure (kills the on-device K projection):
    q k^T = Xi (Wq^T Wk) Xt^T + (per-query-row constants) + 1 (bq^T Wk) Xt^T
Softmax is row-shift invariant, so the row-constant terms drop. With
host-precomputed M = Wq^T Wk and c = bq @ Wk:
    scores ~ (Xi M + 1 c^T) Xt^T
The Q projection becomes A = Xi M + c (same cost, bias reused) and K^T
is just Xt^T — DMA'd once into SBUF and reused both as the scores
stationary and as the V-projection stationary.

DMA strategy: every dma_start costs ~0.7us on the issuing sequencer, so
all bulk tensors are host-relayouted to SBUF-tile order and shipped as
a handful of large contiguous transfers, ordered by first use so the
PE starts ~2us after the NEFF preamble.

Per-core dataflow (matmuls bf16, fp32 PSUM):
  - AT[d',q] per 512-col chunk: stationary M[d,d'_tile], moving XiT.
  - scoresT[k,q] = kt_tile.T @ AT -> exp -> stationary of P@V: the
    2048x2048 probability matrix is never transposed on chip.
  - no-max softmax (scores ~ N(0,1)); row sums via ones-column matmul;
    final normalize fused with bv add in one DVE op per 512-chunk.
"""

import sys
import os

for _p in ("/opt/trn_rl_repo", "/root/.axon_site/_ro/trn_rl_repo"):
    if os.path.isdir(_p) and _p not in sys.path:
        sys.path.insert(0, _p)

import numpy as np
import ml_dtypes

import concourse.bass as bass
import concourse.mybir as mybir
import concourse.tile as tile
from concourse import bacc
from concourse.bass_utils import run_bass_kernel_spmd

BF16 = mybir.dt.bfloat16
F32 = mybir.dt.float32
AF = mybir.ActivationFunctionType
ALU = mybir.AluOpType

B, S, D = 8, 2048, 1024
P = 128
ND = D // P          # 8  d tiles
NS = S // P          # 16 s tiles
QC = 512             # q chunk width (matmul free dim / PSUM bank)
NQC = S // QC        # 4
EC = 512             # e chunk width for V / output
SCALE = 1.0 / float(np.sqrt(D))

_CACHE = {}


def _build_nc():
    nc = bacc.Bacc("TRN2", target_bir_lowering=False, debug=False, num_devices=8)

    # all bulk tensors are pre-relayouted on host to SBUF tile order
    xi_d = nc.dram_tensor("xi", [P, NQC, ND * QC], BF16, kind="ExternalInput").ap()
    xt_d = nc.dram_tensor("xt", [P, ND, S], BF16, kind="ExternalInput").ap()
    m_d = nc.dram_tensor("m", [P, ND, ND * P], BF16, kind="ExternalInput").ap()
    wvt_d = nc.dram_tensor("wvt", [P, ND, D], BF16, kind="ExternalInput").ap()
    ca_d = nc.dram_tensor("ca", [P, ND], F32, kind="ExternalInput").ap()
    bv_d = nc.dram_tensor("bv", [D], F32, kind="ExternalInput").ap()
    out_d = nc.dram_tensor("out", [S, D], F32, kind="ExternalOutput").ap()

    with tile.TileContext(nc) as tc:
        _emit(nc, tc, xi_d, xt_d, m_d, wvt_d, ca_d, bv_d, out_d)
    nc.compile()
    return nc


def _emit(nc, tc, xi_d, xt_d, m_d, wvt_d, ca_d, bv_d, out_d):
    NH = QC // P  # 4 q_tiles per chunk
    with (
        tc.tile_pool(name="const", bufs=1) as pc,
        tc.tile_pool(name="qkv", bufs=1) as pqkv,
    ):
        # persistent activations
        at = pqkv.tile([P, ND, S], BF16, name="at", tag="at")    # AT[d',q]
        kt = pqkv.tile([P, ND, S], BF16, name="kt", tag="kt")    # XtT[d',k]
        # V[s, e] with a ones column appended at e=D: the P@V row sums
        # (softmax denominators) fall out of the third PV chain, so no
        # 1-column rowsum matmuls (whose ldweights never hide) are needed.
        v = pqkv.tile([P, NS, D + 1], BF16, name="v", tag="v")

        # constants
        bias_a = pc.tile([P, ND], F32, name="bias_a", tag="bias_a")
        ones_row = pc.tile([1, P], F32, name="ones_row", tag="ones_row")
        bv_row = pc.tile([1, D], F32, name="bv_row", tag="bv_row")
        bv_bcast = pc.tile([P, D], F32, name="bv_bcast", tag="bv_bcast")


        with (
            tc.tile_pool(name="w", bufs=1) as pw,
            tc.tile_pool(name="xs", bufs=1) as pxs,
            tc.tile_pool(name="psP", bufs=8, space="PSUM") as psP,
        ):
            # m_sb[:, et, d*P:(d+1)*P] = M[d-block, et-block] (et-major!)
            m_sb = pw.tile([P, ND, ND * P], BF16, name="m_sb", tag="m_sb")
            wv_sb = pw.tile([P, ND, D], BF16, name="wv_sb", tag="wv_sb")
            # xc[:, qc, d*QC:(d+1)*QC] = XiT[d-block, qc-chunk]
            xc = pxs.tile([P, NQC, ND * QC], BF16, name="xc", tag="xs")

            # --- bulk DMAs: few triggers, ordered by first use.
            # Ring order is per-engine FIFO; active rings share the 16 SDMA
            # engines, so first-needed pieces are kept small and bulk (kt,
            # wv) is queued last on each ring.
            # NOTE: gpsimd triggers lower to qPoolDynamic (the VECTOR
            # engine's stream, slow SW completions) — avoid it entirely;
            # everything rides the two HWDGE rings (sync/scalar).
            nc.scalar.dma_start(bv_row[:], bv_d[None, :])         # 4KB
            nc.sync.dma_start(bias_a[:], ca_d[:])                 # 4KB
            nc.sync.dma_start(m_sb[:, 0, 0:4 * P], m_d[:, 0, 0:4 * P])  # 128K
            nc.scalar.dma_start(xc[:, 0, 0:4 * QC], xi_d[:, 0, 0:4 * QC])
            nc.sync.dma_start(m_sb[:, 0, 4 * P:], m_d[:, 0, 4 * P:])
            nc.scalar.dma_start(xc[:, 0, 4 * QC:], xi_d[:, 0, 4 * QC:])
            nc.sync.dma_start(m_sb[:, 1, :], m_d[:, 1, :])
            nc.sync.dma_start(m_sb[:, 2:5, :], m_d[:, 2:5, :])    # 768KB
            nc.sync.dma_start(m_sb[:, 5:8, :], m_d[:, 5:8, :])
            nc.scalar.dma_start(xc[:, 1, :], xi_d[:, 1, :])
            nc.scalar.dma_start(xc[:, 2, :], xi_d[:, 2, :])
            nc.scalar.dma_start(xc[:, 3, :], xi_d[:, 3, :])
            # bulk not needed until the V phase (~70us), at the ring tails
            nc.sync.dma_start(wv_sb[:], wvt_d[:])                 # 2MB
            nc.sync.dma_start(kt[:, 0:4, :], xt_d[:, 0:4, :])
            nc.scalar.dma_start(kt[:, 4:8, :], xt_d[:, 4:8, :])
            nc.vector.memset(ones_row[:], 1.0)
            nc.vector.memset(v[:, :, D:D + 1], 1.0)

            # bv broadcast first: needs only bv_row + memset, warms the PE
            for c in range(2):
                pb = psP.tile([P, EC], F32, name="pb", tag="ps")
                nc.tensor.matmul(
                    pb[:], ones_row[:], bv_row[:, c * EC:(c + 1) * EC],
                    start=True, stop=True)
                nc.vector.tensor_copy(bv_bcast[:, c * EC:(c + 1) * EC], pb[:])

            # --- AT[d'_t, qc] = sum_d M[d, d'_t].T @ XiT[d, qc]  (+ c) ---
            for qc in range(NQC):
                for et in range(ND):
                    ps = psP.tile([P, QC], F32, name="ps", tag="ps")
                    for d in range(ND):
                        nc.tensor.matmul(
                            ps[:], m_sb[:, et, d * P:(d + 1) * P],
                            xc[:, qc, d * QC:(d + 1) * QC],
                            start=(d == 0), stop=(d == ND - 1))
                    nc.vector.tensor_scalar_add(
                        at[:, et, qc * QC:(qc + 1) * QC], ps[:],
                        bias_a[:, et:et + 1])

            # --- V[s_t, e] = sum_d XtT[d, s_t].T @ Wv^T[d, e] ---
            for vc in range(NQC):
                for si in range(NH):
                    st = vc * NH + si
                    ps0 = psP.tile([P, EC], F32, name="ps0", tag="ps")
                    ps1 = psP.tile([P, EC], F32, name="ps1", tag="ps")
                    for d in range(ND):
                        lhs = kt[:, d, st * P:(st + 1) * P]
                        nc.tensor.matmul(ps0[:], lhs, wv_sb[:, d, 0:EC],
                                         start=(d == 0), stop=(d == ND - 1))
                        nc.tensor.matmul(ps1[:], lhs, wv_sb[:, d, EC:D],
                                         start=(d == 0), stop=(d == ND - 1))
                    nc.vector.tensor_copy(v[:, st, 0:EC], ps0[:])
                    nc.vector.tensor_copy(v[:, st, EC:D], ps1[:])

        # --- attention ---
        with (
            tc.tile_pool(name="et", bufs=3) as pet,
            tc.tile_pool(name="outp", bufs=2) as pout,
            tc.tile_pool(name="stat", bufs=4) as pstat,
            tc.tile_pool(name="psST", bufs=2, space="PSUM") as psST,
            tc.tile_pool(name="psAV", bufs=6, space="PSUM") as psAV,
        ):
            for qc in range(NQC):
                # scores^T for this q chunk: ET[kk, q] = exp(scale*XtT.T@AT)
                et_t = pet.tile([P, NS, QC], BF16, name="et_t", tag="et")
                for kk in range(NS):
                    st_ps = psST.tile([P, QC], F32, name="st_ps", tag="st")
                    for e in range(ND):
                        nc.tensor.matmul(
                            st_ps[:],
                            kt[:, e, kk * P:(kk + 1) * P],
                            at[:, e, qc * QC:(qc + 1) * QC],
                            start=(e == 0), stop=(e == ND - 1))
                    nc.scalar.activation(et_t[:, kk, :], st_ps[:], AF.Exp,
                                         scale=SCALE)

                # attended[q_t, :] = (ET.T @ [V|1]) * recip + bv
                # three chains per q_tile: cols 0:512, 512:768, 768:1025
                # (last includes the ones column => softmax denominators).
                # Every chain streams >=256 rows, so each matmul's
                # ldweights hides under the previous stream.
                C1, C2 = 512, 768
                for qs in range(NH):
                    q_tile = qc * NH + qs
                    last = q_tile == NS - 1
                    a0 = psAV.tile([P, EC], F32, name="a0", tag="av")
                    a1 = psAV.tile([P, EC], F32, name="a1", tag="av")
                    a2 = psAV.tile([P, EC], F32, name="a2", tag="av")
                    recip = pstat.tile([P, 1], F32, name="recip", tag="recip")
                    ob = pout.tile([P, D], F32, name="ob", tag="ob")
                    if not last:
                        for kk in range(NS):
                            lhs = et_t[:, kk, qs * P:(qs + 1) * P]
                            nc.tensor.matmul(a0[:], lhs, v[:, kk, 0:C1],
                                             start=(kk == 0),
                                             stop=(kk == NS - 1))
                            nc.tensor.matmul(a1[:, 0:C2 - C1], lhs,
                                             v[:, kk, C1:C2],
                                             start=(kk == 0),
                                             stop=(kk == NS - 1))
                            nc.tensor.matmul(a2[:, 0:D + 1 - C2], lhs,
                                             v[:, kk, C2:D + 1],
                                             start=(kk == 0),
                                             stop=(kk == NS - 1))
                        nc.vector.reciprocal(recip[:], a2[:, D - C2:D - C2 + 1])
                        nc.vector.scalar_tensor_tensor(
                            ob[:, 0:C1], a0[:], recip[:], bv_bcast[:, 0:C1],
                            op0=ALU.mult, op1=ALU.add)
                        nc.vector.scalar_tensor_tensor(
                            ob[:, C1:C2], a1[:, 0:C2 - C1], recip[:],
                            bv_bcast[:, C1:C2],
                            op0=ALU.mult, op1=ALU.add)
                        nc.vector.scalar_tensor_tensor(
                            ob[:, C2:D], a2[:, 0:D - C2], recip[:],
                            bv_bcast[:, C2:D],
                            op0=ALU.mult, op1=ALU.add)
                        eng = nc.sync if q_tile % 2 == 0 else nc.scalar
                        eng.dma_start(
                            out_d[q_tile * P:(q_tile + 1) * P, :], ob[:])
                    else:
                        # final tile: upper cols (+denominator) first so
                        # their combine+DMA overlap the cols 0:512 chain —
                        # the tail after the last matmul is small.
                        for kk in range(NS):
                            lhs = et_t[:, kk, qs * P:(qs + 1) * P]
                            nc.tensor.matmul(a1[:, 0:C2 - C1], lhs,
                                             v[:, kk, C1:C2],
                                             start=(kk == 0),
                                             stop=(kk == NS - 1))
                            nc.tensor.matmul(a2[:, 0:D + 1 - C2], lhs,
                                             v[:, kk, C2:D + 1],
                                             start=(kk == 0),
                                             stop=(kk == NS - 1))
                        nc.vector.reciprocal(recip[:], a2[:, D - C2:D - C2 + 1])
                        nc.vector.scalar_tensor_tensor(
                            ob[:, C1:C2], a1[:, 0:C2 - C1], recip[:],
                            bv_bcast[:, C1:C2],
                            op0=ALU.mult, op1=ALU.add)
                        nc.vector.scalar_tensor_tensor(
                            ob[:, C2:D], a2[:, 0:D - C2], recip[:],
                            bv_bcast[:, C2:D],
                            op0=ALU.mult, op1=ALU.add)
                        nc.scalar.dma_start(
                            out_d[q_tile * P:(q_tile + 1) * P, C1:D],
                            ob[:, C1:D])
                        for kk in range(NS):
                            lhs = et_t[:, kk, qs * P:(qs + 1) * P]
                            nc.tensor.matmul(a0[:], lhs, v[:, kk, 0:C1],
                                             start=(kk == 0),
                                             stop=(kk == NS - 1))
                        nc.vector.scalar_tensor_tensor(
                            ob[:, 0:C1 // 2], a0[:, 0:C1 // 2], recip[:],
                            bv_bcast[:, 0:C1 // 2],
                            op0=ALU.mult, op1=ALU.add)
                        nc.sync.dma_start(
                            out_d[q_tile * P:(q_tile + 1) * P, 0:C1 // 2],
                            ob[:, 0:C1 // 2])
                        nc.vector.scalar_tensor_tensor(
                            ob[:, C1 // 2:C1], a0[:, C1 // 2:C1], recip[:],
                            bv_bcast[:, C1 // 2:C1],
                            op0=ALU.mult, op1=ALU.add)
                        nc.scalar.dma_start(
                            out_d[q_tile * P:(q_tile + 1) * P, C1 // 2:C1],
                            ob[:, C1 // 2:C1])


def get_nc():
    if "nc" not in _CACHE:
        _CACHE["nc"] = _build_nc()
    return _CACHE["nc"]


def _prep_inputs(image_emb, text_emb, Wq, bq, Wk, bk, Wv, bv):
    bf = ml_dtypes.bfloat16
    xi = np.asarray(image_emb)   # [B, S, D] f32
    xt = np.asarray(text_emb)
    wq = np.asarray(Wq, dtype=np.float32)
    wk = np.asarray(Wk, dtype=np.float32)

    # m host layout [P, ND(et), ND(d)*P]: m[p, et, d*P+c] = M[d*P+p, et*P+c]
    m = (wq.T @ wk).astype(bf)                       # [D, D]
    m = m.reshape(ND, P, ND, P).transpose(1, 2, 0, 3).reshape(P, ND, ND * P)
    m = np.ascontiguousarray(m)

    ca = np.asarray(bq, dtype=np.float32) @ wk       # [D]
    ca = np.ascontiguousarray(ca.reshape(ND, P).T)   # [P, ND]

    # wvt [P, ND(d), D(e)]: wvt[p, d, e] = Wv[e, d*P+p]
    wvt = np.asarray(Wv).T.astype(bf).reshape(ND, P, D).transpose(1, 0, 2)
    wvt = np.ascontiguousarray(wvt)

    # xt [B, P, ND(d), S]: XtT tile order
    xtT = xt.transpose(0, 2, 1).astype(bf)           # [B, D, S]
    xtr = np.ascontiguousarray(
        xtT.reshape(B, ND, P, S).transpose(0, 2, 1, 3))

    # xi [B, P, NQC, ND*QC]: xi[b, p, qc, d*QC+c] = XiT[b, d*P+p, qc*QC+c]
    xiT = xi.transpose(0, 2, 1).astype(bf)           # [B, D, S]
    xir = np.ascontiguousarray(
        xiT.reshape(B, ND, P, NQC, QC).transpose(0, 2, 3, 1, 4)
        .reshape(B, P, NQC, ND * QC))

    bv = np.asarray(bv, dtype=np.float32)
    in_maps = []
    for b in range(B):
        in_maps.append({
            "xi": xir[b], "xt": xtr[b],
            "m": m, "wvt": wvt, "ca": ca, "bv": bv,
        })
    return in_maps


def run(image_emb, text_emb, Wq, bq, Wk, bk, Wv, bv, trace=False, **spmd_kwargs):
    nc = get_nc()
    in_maps = _prep_inputs(image_emb, text_emb, Wq, bq, Wk, bk, Wv, bv)
    res = run_bass_kernel_spmd(nc, in_maps, list(range(B)), trace=trace,
                               **spmd_kwargs)
    out = np.stack([res.results[b]["out"] for b in range(B)], axis=0)
    return out, res


def kernel(image_emb, text_emb, edge_index=None, Wq=None, bq=None, Wk=None,
           bk=None, Wv=None, bv=None, **_unused):
    out, _ = run(image_emb, text_emb, Wq, bq, Wk, bk, Wv, bv, trace=False)
    return out
